# revision 40
# baseline (speedup 1.0000x reference)
"""FlowNetC correlation (kernel_size=1, max_disp=20, stride2=2) on 8 Trainium2 cores.

Problem: inputs input1, input2 of shape [8, 256, 64, 96] fp32; output
[8, 441, 64, 96] fp32 with
  out[b, i*21+j, y, x] = (1/256) * sum_c in1[b,c,y,x] * in2[b,c,y+2i-20,x+2j-20]
(zero where the in2 index is out of range).

Sharding: data-parallel over batch - core b handles batch element b.

Per-core strategy: tile (y, x) into 48 parity-separated blocks of
BY*BX = 32*4 = 128 pixels (partition m = 4*b + a; b = y-index, a = x-index).
With 32 stride-2 rows the r-window of every block is exactly the 32
same-parity image rows (no clipping), and the u-window is 14-24 wide
(~the 21 valid).  Per block the TensorEngine computes the banded product
P[m, rr, c] = sum_ch in1[ch, pixel m] * in2[ch, r(rr), u(c)] as two
[128 x (16*nu)] fp16 matmuls per 128-channel chunk (fp32 PSUM accumulate
over the two chunks).  ACT/DVE drain the two PSUM chunks into a per-quad
SBUF tile st[128, 32 rows, 4 subblocks, nu] (rows contiguous).

Output write is r-compacted: pixel (b) only needs band rows
[dr(b), dr(b)+21), dr(b) = clamp(b-10, 0, 11).  Partitions are split into
3 contiguous b-ranges {12,8,12}; each group ships one contiguous row slab
[o_g, o_g+rows_g) covering all its pixels' windows (one 2-D DMA per
(quad, group), innermost run = rows_g*4*nu fp16 >= 1.2 KB, full DMA rate).
This cuts the band write from 8.3 MB to 6.06 MB; total DMA traffic is
12.35 MB/core (inputs 2 x 3.15 MB fp16 + output), ~34.3 us busy at the
360 GB/s cost-model aggregate, with the PE stream (64.5k cycles ~= 27 us)
hidden under it.  The host scatters the shipped slabs into the final
output via a precomputed index map.

Scheduling notes (cost-model timeline): input pieces alternate between SP
HWDGE and Pool SWDGE (one descriptor generator alone cannot feed the DMA
engines); in2 is parity- and x-major so pieces arrive in the order quads
consume them; quad writes trail compute by 3 quads so they never steal
input bandwidth while PE is input-paced; output DMAs issue only from SP
and Pool so a cross-engine wait can never head-of-line-block an ACT/DVE
drain.  Measured: 40562 ns/core vs 44302 ns for the previous kernel.
"""

import numpy as np

C, H, W = 256, 64, 96
D = 21
PADV = 20
B = 8
N_CORES = 8
BY, BX = 32, 4
NQ = 12  # quads (x0-groups); blocks = NQ * 4 parities
NBLK = 48
QSTRIDE = 512  # psum bank size in fp32 elements

# contiguous b-group split for the r-compacted writes: (b_lo, b_hi, o_g, rows_g)
# 3 groups balance write bytes (3008 partition-rows vs 4096 full) against
# DMA-issue count (36 output DMAs)
GROUPS = [(0, 12, 0, 22), (12, 20, 2, 28), (20, 32, 10, 22)]


def _quad_geometry():
    """Per quad q (x0 = 8q): u columns per px parity and nu (parity-invariant)."""
    quads = []
    for q in range(NQ):
        x0 = 8 * q
        us = {}
        for px in (0, 1):
            u_lo = x0 + px - PADV
            while u_lo < 0:
                u_lo += 2
            u_hi = min(x0 + px + 2 * (BX - 1) + PADV, W - 1)
            us[px] = list(range(u_lo, u_hi + 1, 2))
        assert len(us[0]) == len(us[1])
        quads.append(dict(x0=x0, us=us, nu=len(us[0])))
    return quads


_QUADS = _quad_geometry()
_GATHER = None
_PROGRAM = None

# pixel coordinates per block: blk = q*4 + py*2 + px, m = 4*b + a
_YM = np.zeros((NBLK, 128), dtype=np.int64)
_XM = np.zeros((NBLK, 128), dtype=np.int64)
for _q in range(NQ):
    for _py in (0, 1):
        for _px in (0, 1):
            _blk = _q * 4 + _py * 2 + _px
            _b = np.arange(128) // 4
            _a = np.arange(128) % 4
            _YM[_blk] = _py + 2 * _b
            _XM[_blk] = 8 * _q + _px + 2 * _a


def _out_layout():
    """Flat fp16 output: per (quad, group) a [parts, rows, 4, nu] slab."""
    offs = []
    off = 0
    for q in range(NQ):
        nu = _QUADS[q]["nu"]
        for lo, hi, og, rows in GROUPS:
            parts = 4 * (hi - lo)
            offs.append((q, lo, hi, og, rows, off))
            off += parts * rows * 4 * nu
    return offs, off


_OUT_OFFS, _OUT_TOTAL = _out_layout()


def _build_gather():
    """Indices such that O.flat[dst] = R_flat[src] for one core's output."""
    dst_list, src_list = [], []
    for q, lo, hi, og, rows, off in _OUT_OFFS:
        nu = _QUADS[q]["nu"]
        parts = 4 * (hi - lo)
        s = np.arange(parts)
        m = 4 * lo + s
        b = m // 4
        a = m % 4
        i_s = np.arange(rows)
        p4 = np.arange(4)
        py = p4 // 2
        px = p4 % 2
        c = np.arange(nu)
        # broadcast [parts, rows, 4, nu]
        bb = b[:, None, None, None]
        aa = a[:, None, None, None]
        rr = (og + i_s)[None, :, None, None]
        pyy = py[None, None, :, None]
        pxx = px[None, None, :, None]
        cc = c[None, None, None, :]
        y = pyy + 2 * bb
        u0 = np.array([[_QUADS[q]["us"][0][0], _QUADS[q]["us"][1][0]]])
        x = 8 * q + pxx + 2 * aa
        u = u0[0][p4 % 2][None, None, :, None] + 2 * cc
        i = rr - bb + 10
        j = (u - x + PADV) // 2
        valid = (i >= 0) & (i < D) & (j >= 0) & (j < D)
        d = i * D + j
        dst = (d * H + y) * W + x
        src = off + ((s[:, None, None, None] * rows + (rr - og)) * 4 + p4[None, None, :, None]) * nu + cc
        bcast = np.broadcast_arrays(dst, src, valid)
        dst_list.append(bcast[0][bcast[2]])
        src_list.append(bcast[1][bcast[2]])
    dst = np.concatenate(dst_list)
    src = np.concatenate(src_list)
    # every output entry that is reachable (valid displacement) must be covered
    assert len(np.unique(dst)) == len(dst)
    return dst, src


def _gather_indices():
    global _GATHER
    if _GATHER is None:
        _GATHER = _build_gather()
    return _GATHER


def _build_program():
    from contextlib import ExitStack

    import concourse.bacc as bacc
    import concourse.mybir as mybir
    import concourse.tile as tile

    in_dt = mybir.dt.float16
    out_dt = mybir.dt.float16

    nc = bacc.Bacc("TRN2", target_bir_lowering=False, debug=False)
    # in1 pre-packed on host: [p, kc, blk, m] = in1[kc*128+p, YM[blk,m], XM[blk,m]]
    in1_d = nc.dram_tensor("in1", [128, 2, NBLK, 128], in_dt, kind="ExternalInput")
    # in2 parity- and x-major: [p, kc, py, x, k] = in2[kc*128+p, py+2k, x]
    # (x-major so input pieces can split along x, shrinking the first-drain gate)
    in2_d = nc.dram_tensor("in2", [128, 2, 2, W, 32], in_dt, kind="ExternalInput")
    out_d = nc.dram_tensor("out", [_OUT_TOTAL], out_dt, kind="ExternalOutput")

    with ExitStack() as ctx:
        tc = ctx.enter_context(tile.TileContext(nc))
        inp_pool = ctx.enter_context(tc.tile_pool(name="inp", bufs=1))
        psum_pool = ctx.enter_context(tc.tile_pool(name="psum", bufs=4, space="PSUM"))
        out_pool = ctx.enter_context(tc.tile_pool(name="outp", bufs=8))

        in1_s = inp_pool.tile([128, 2, NBLK, 128], in_dt)
        in2_s = inp_pool.tile([128, 2, 2, W, 32], in_dt)

        def load_in1(kc, q0, q1, eng=None):
            # kc=None loads both halves in one DMA (3-dim AP, 1 KB+ runs)
            k = slice(None) if kc is None else kc
            (eng or nc.gpsimd).dma_start(
                in1_s[:, k, 4 * q0 : 4 * q1, :], in1_d[:, k, 4 * q0 : 4 * q1, :]
            )

        def load_in2(kc, py, x0, x1, eng=None):
            k = slice(None) if kc is None else kc
            sl = slice(x0, x1)
            (eng or nc.gpsimd).dma_start(
                in2_s[:, k, py, sl], in2_d[:, k, py, sl]
            )

        # enqueue all input pieces up front; emission order sets priority.
        # in2 splits into x-ranges sized to when quads need them (quad q needs
        # x <= 8q+27), so the first-drain gate is only ~0.72 MB.  Pieces
        # alternate between SP HWDGE and Pool SWDGE early on: each generator
        # takes ~1-1.3 us per descriptor set, and one alone cannot feed the
        # DMA engines — two in parallel can.
        loads = [
            lambda e: load_in1(0, 0, 1, eng=e),
            lambda e: load_in2(0, 0, 0, 28, eng=e),
            lambda e: load_in1(1, 0, 1, eng=e),
            lambda e: load_in2(1, 0, 0, 28, eng=e),
            lambda e: load_in1(0, 1, 2, eng=e),
            lambda e: load_in2(0, 0, 28, 48, eng=e),
            lambda e: load_in1(1, 1, 2, eng=e),
            lambda e: load_in2(1, 0, 28, 48, eng=e),
            lambda e: load_in1(0, 2, 3, eng=e),
            lambda e: load_in2(0, 1, 0, 48, eng=e),
            lambda e: load_in1(1, 2, 3, eng=e),
            lambda e: load_in2(1, 1, 0, 48, eng=e),
            lambda e: load_in1(0, 3, 6, eng=e),
            lambda e: load_in2(0, 0, 48, 96, eng=e),
            lambda e: load_in2(1, 0, 48, 96, eng=e),
            lambda e: load_in1(1, 3, 6, eng=e),
            lambda e: load_in2(0, 1, 48, 96, eng=e),
            lambda e: load_in2(1, 1, 48, 96, eng=e),
            lambda e: load_in1(0, 6, 9, eng=e),
            lambda e: load_in1(1, 6, 9, eng=e),
            lambda e: load_in1(0, 9, 12, eng=e),
            lambda e: load_in1(1, 9, 12, eng=e),
        ]
        # alternate only the early (prep-rate-critical) pieces; the tail
        # stays on Pool so late HWDGE pieces can't jump the priority order
        for i, ld in enumerate(loads):
            ld(nc.sync if (i < 13 and i % 2 == 0) else nc.gpsimd)

        # half-pass block order: all py=0 blocks of a 6-quad half first, then
        # the py=1 blocks (quad writes unlock after py=1) — PE never waits on
        # the in2 py=1 pieces and quad writes start flowing ~mid-run
        stvs = {}

        def do_block(q, py, px):
            g = _QUADS[q]
            nu = g["nu"]
            if q not in stvs:
                st_flat = out_pool.tile([128, 32 * 4 * 24], out_dt, tag="st")
                stvs[q] = st_flat[:, : 32 * 4 * nu].rearrange(
                    "p (r f n) -> p r f n", r=32, f=4, n=nu
                )
            stv = stvs[q]
            blk = q * 4 + py * 2 + px
            p4 = py * 2 + px
            u_lo = g["us"][px][0]
            pt = psum_pool.tile([128, 2, QSTRIDE], mybir.dt.float32, tag="pt")
            # uneven 18/14 row split: ACT (faster per element) drains the
            # larger chunk, DVE the smaller — balances the two drain engines
            for c, (r0, r1) in enumerate(((0, 18), (18, 32))):
                for kc in (0, 1):
                    lhsT = in1_s[:, kc, blk, :]
                    # x-major in2: transpose free dims so psum stays r-major
                    rhs = in2_s[
                        :, kc, py, u_lo : u_lo + 2 * nu - 1 : 2, r0:r1
                    ].transpose([0, 2, 1])
                    nc.tensor.matmul(
                        pt[:, c, : (r1 - r0) * nu],
                        lhsT,
                        rhs,
                        start=(kc == 0),
                        stop=(kc == 1),
                    )
            nc.scalar.copy(stv[:, 0:18, p4, :], pt[:, 0, : 18 * nu])
            nc.vector.tensor_copy(stv[:, 18:32, p4, :], pt[:, 1, : 14 * nu])

        def write_quad(q):
            # r-compacted group writes, one DMA per partition group. Only SP
            # (HWDGE) and Pool (SWDGE) issue them: ACT/DVE stay drain-only so
            # a cross-engine DMA wait can never head-of-line-block a drain.
            nu = _QUADS[q]["nu"]
            stv = stvs.pop(q)
            ng = len(GROUPS)
            # the very last quad has no later drains, so ACT can safely take
            # one of its writes
            engines = (
                (nc.sync, nc.scalar, nc.gpsimd)
                if q == NQ - 1
                else (nc.sync, nc.gpsimd, nc.sync)
            )
            for eng, (q2, lo, hi, og, rows, off) in zip(
                engines, _OUT_OFFS[q * ng : q * ng + ng]
            ):
                parts = 4 * (hi - lo)
                n = parts * rows * 4 * nu
                src = stv[4 * lo : 4 * hi, og : og + rows, :, :]
                dst = out_d[off : off + n].rearrange("(p r) -> p r", p=parts)
                eng.dma_start(dst, src)

        # First triplet runs its py=0 blocks before any py=1 block (the in2
        # py=1 pieces land ~7 us in); from q3 on the pieces are all resident.
        # Writes trail their quad by 2 so they don't steal DMA bandwidth from
        # the input stream while PE is still input-paced.
        for q in (0, 1, 2):
            do_block(q, 0, 0)
            do_block(q, 0, 1)
        for q in (0, 1, 2):
            do_block(q, 1, 0)
            do_block(q, 1, 1)
        # write delay shrinks from 3 quads to 1 near the end, so only the
        # final quad's write remains after the last compute
        wq = iter(range(NQ))
        for q in range(3, NQ):
            for py in (0, 1):
                for px in (0, 1):
                    do_block(q, py, px)
            write_quad(next(wq))
            if q >= 10:
                write_quad(next(wq))
        write_quad(next(wq))

    nc.compile()
    return nc


def _program():
    global _PROGRAM
    if _PROGRAM is None:
        _PROGRAM = _build_program()
    return _PROGRAM


def _prep_in1(x):
    # [256, 64, 96] -> [128, 2, NBLK, 128] block-packed
    x2 = x.reshape(2, 128, H, W)
    g = x2[:, :, _YM, _XM]  # [2, 128, NBLK, 128]
    return np.ascontiguousarray(g.transpose(1, 0, 2, 3), dtype=np.float16)


def _prep_in2(x):
    # [256, 64, 96] -> [128, 2, 2, 96, 32]: [p, kc, py, x, k] = in2[kc*128+p, py+2k, x]
    g = x.reshape(2, 128, 32, 2, W).transpose(1, 0, 3, 4, 2)
    return np.ascontiguousarray(g, dtype=np.float16)


def make_in_maps(input1, input2):
    in1 = np.asarray(input1, dtype=np.float32)
    in2 = np.asarray(input2, dtype=np.float32)
    return [
        {"in1": _prep_in1(in1[b]), "in2": _prep_in2(in2[b])} for b in range(B)
    ]


def extract_output(R):
    """R: [_OUT_TOTAL] device result -> [441, 64, 96] fp32."""
    dst, src = _gather_indices()
    O = np.zeros(D * D * H * W, dtype=np.float32)
    O[dst] = R.reshape(-1)[src].astype(np.float32)
    O *= np.float32(1.0 / C)
    return O.reshape(D * D, H, W)


def run_spmd(in_maps, **kwargs):
    from concourse import bass_utils

    return bass_utils.run_bass_kernel_spmd(
        _program(), in_maps, core_ids=list(range(N_CORES)), **kwargs
    )


def kernel(input1, input2):
    in_maps = make_in_maps(input1, input2)
    res = run_spmd(in_maps)
    return np.stack([extract_output(res.results[b]["out"]) for b in range(B)])
